# revision 21
# baseline (speedup 1.0000x reference)
"""MoE FFN (B=4, S=2048, D=1024, H=4096, E=8, top-2) on 8 NeuronCores.

Strategy: expert parallelism. The router (tiny: 134 MFLOP) runs on host as
part of input sharding; tokens are dispatched (gathered) per expert on host,
one expert per core. Each core runs a batched SwiGLU FFN over its expert's
tokens in feature-major layout (tokens on the free axis), so the natural
numpy weight layouts Wg/Wu [D,H] and Wd [H,D] are directly the matmul
stationary operands and no transposes are needed anywhere. Outputs are
combined (weighted scatter-add) on host in the same expert order as the
reference.

Per core: Wg/Wu (bf16) stay resident in SBUF (128 KB/partition), Wd is
streamed per token group, activations flow x -> gate/up (PSUM) -> swiglu
(ACT+DVE) -> h (SBUF bf16) -> down proj (PSUM) -> y (fp32).
"""

import sys

for _p in ("/opt/trn_rl_repo", "/root/.axon_site/_ro/trn_rl_repo"):
    if _p not in sys.path:
        sys.path.append(_p)

import numpy as np
import ml_dtypes

import concourse.bacc as bacc
import concourse.mybir as mybir
import concourse.tile as tile
from concourse.bass_utils import run_bass_kernel_spmd

# If this environment lacks antenv.axon_hooks, run_bass_kernel_spmd's
# trace=True path (e.g. via BASS_TRACE=1) would die on import. Register a
# stub that reports "no hook" so tracing degrades gracefully; a real
# antenv.axon_hooks, when present, is untouched.
try:
    import antenv.axon_hooks  # noqa: F401
except ImportError:
    import types
    import antenv
    _stub = types.ModuleType("antenv.axon_hooks")
    _stub.get_axon_ntff_profile_hook = lambda: None
    sys.modules["antenv.axon_hooks"] = _stub
    antenv.axon_hooks = _stub

BF16 = mybir.dt.bfloat16
F32 = mybir.dt.float32

B, S, D = 4, 2048, 1024
H = 4096
E = 8
TOP_K = 2
N_CORES = 8
P = 128          # partitions
G_FULL = 512     # token group (matmul free dim)
KD = D // P      # 8  k-tiles over D
KH = H // P      # 32 k-tiles over H
MH = H // P      # 32 m-tiles over H
MD = D // P      # 8  m-tiles over D


def _token_groups(C):
    """Split C tokens into groups of 512 plus at most one 128..384 tail."""
    groups = []
    off = 0
    while C - off >= G_FULL:
        groups.append((off, G_FULL))
        off += G_FULL
    if C - off > 0:
        groups.append((off, C - off))
        off = C
    return groups


def build_expert_ffn(C, _max_groups=None):
    """One-expert batched SwiGLU FFN over C tokens (feature-major)."""
    nc = bacc.Bacc("TRN2", target_bir_lowering=False, debug=False,
                   num_devices=N_CORES)
    xT = nc.dram_tensor("xT", [D, C], BF16, kind="ExternalInput")
    wg_d = nc.dram_tensor("Wg", [D, H], BF16, kind="ExternalInput")
    wu_d = nc.dram_tensor("Wu", [D, H], BF16, kind="ExternalInput")
    wd_d = nc.dram_tensor("Wd", [H, D], BF16, kind="ExternalInput")
    yT = nc.dram_tensor("yT", [D, C], F32, kind="ExternalOutput")

    groups = _token_groups(C)
    if _max_groups is not None:
        groups = groups[:_max_groups]

    with tile.TileContext(nc) as tc:
        with (
            tc.tile_pool(name="wconst", bufs=1) as wconst,
            tc.tile_pool(name="wdp", bufs=12) as wdp,
            tc.tile_pool(name="xp", bufs=12) as xp,
            tc.tile_pool(name="hp", bufs=32) as hp,
            tc.tile_pool(name="accp", bufs=8, space="PSUM") as accp,
            tc.tile_pool(name="tmpp", bufs=2) as tmpp,
            tc.tile_pool(name="ysp", bufs=3) as ysp,
        ):
            def load_x_group(g0, G):
                x_t = []
                for k in range(KD):
                    xt = xp.tile([P, G_FULL], BF16, tag="xt", name=f"xt{k}")
                    nc.sync.dma_start(
                        out=xt[:, :G],
                        in_=xT[k * P:(k + 1) * P, g0:g0 + G])
                    x_t.append(xt)
                return x_t

            # resident Wg/Wu, tiled [128, WCHUNK] per (k-strip, column chunk).
            # Separate tiles per chunk give fine-grained deps, and c-major
            # emission order means pass A's m-tile m only waits for chunk
            # m//4 of the stream, not the whole 16MB preload. DMAs are FIFO,
            # so emission order is load-bearing: interleave chunk 0 of Wg
            # with group 0's tokens so the first matmuls' deps land first.
            WCHUNK = 512
            NCH = H // WCHUNK
            wg_t = [[None] * NCH for _ in range(KD)]
            wu_t = [[None] * NCH for _ in range(KD)]

            def load_w_chunk(tiles, dram, pfx, c):
                for k in range(KD):
                    t = wconst.tile([P, WCHUNK], BF16, tag=f"{pfx}{k}_{c}",
                                    name=f"{pfx}{k}_{c}")
                    nc.sync.dma_start(
                        out=t[:],
                        in_=dram[k * P:(k + 1) * P,
                                 c * WCHUNK:(c + 1) * WCHUNK])
                    tiles[k][c] = t

            # interleave group-0 tokens with Wg chunk 0 per-k: the first
            # gate accumulation needs exactly these 16 transfers, so pair
            # them instead of serializing one stream behind the other
            g00, G0 = groups[0]
            x_t0 = []
            for k in range(KD):
                tg = wconst.tile([P, WCHUNK], BF16, tag=f"wg{k}_0",
                                 name=f"wg{k}_0")
                nc.sync.dma_start(out=tg[:],
                                  in_=wg_d[k * P:(k + 1) * P, 0:WCHUNK])
                wg_t[k][0] = tg
                xt = xp.tile([P, G_FULL], BF16, tag="xt", name=f"xt{k}")
                nc.sync.dma_start(out=xt[:, :G0],
                                  in_=xT[k * P:(k + 1) * P, g00:g00 + G0])
                x_t0.append(xt)
            load_w_chunk(wu_t, wu_d, "wu", 0)
            for c in range(1, NCH):
                load_w_chunk(wg_t, wg_d, "wg", c)
                load_w_chunk(wu_t, wu_d, "wu", c)

            for gi, (g0, G) in enumerate(groups):
                x_t = x_t0 if gi == 0 else load_x_group(g0, G)

                # pass A: h = silu(x @ Wu) * (x @ Wg), feature-major
                h_t = []
                for m in range(MH):
                    gate_ps = accp.tile([P, G_FULL], F32, tag="acc",
                                        name=f"gate{m}")
                    up_ps = accp.tile([P, G_FULL], F32, tag="acc",
                                      name=f"up{m}")
                    mc, mo = divmod(m * P, WCHUNK)
                    for k in range(KD):
                        nc.tensor.matmul(
                            gate_ps[:, :G],
                            lhsT=wg_t[k][mc][:, mo:mo + P],
                            rhs=x_t[k][:, :G],
                            start=(k == 0), stop=(k == KD - 1))
                    for k in range(KD):
                        nc.tensor.matmul(
                            up_ps[:, :G],
                            lhsT=wu_t[k][mc][:, mo:mo + P],
                            rhs=x_t[k][:, :G],
                            start=(k == 0), stop=(k == KD - 1))
                    smid = tmpp.tile([P, G_FULL], F32, tag="smid",
                                     name=f"smid{m}")
                    nc.scalar.activation(smid[:, :G], up_ps[:, :G],
                                         mybir.ActivationFunctionType.Silu)
                    ht = hp.tile([P, G_FULL], BF16, tag="h", name=f"h{m}")
                    nc.vector.tensor_mul(ht[:, :G], smid[:, :G],
                                         gate_ps[:, :G])
                    h_t.append(ht)

                # pass B: y = h @ Wd (Wd streamed; A and B time-share the
                # 8 PSUM banks via the shared "acc" tag). Normally one kh
                # sweep over all 8 md tiles; for the LAST group, two md-sets
                # of 4 so the first half's outputs drain to DRAM while the
                # second half still computes (shortens the kernel tail).
                # Finer splits lose: fewer MMs per kh makes the sweep
                # DMA-latency-bound.
                md_sets = ([list(range(MD))] if gi < len(groups) - 1
                           else [[0, 1, 2, 3], [4, 5, 6, 7]])
                for mds in md_sets:
                    y_ps = {md: accp.tile([P, G_FULL], F32, tag="acc",
                                          name=f"y{md}") for md in mds}
                    for kh in range(KH):
                        wdt = wdp.tile([P, D], BF16, tag="wd",
                                       name=f"wd{kh}")
                        nc.sync.dma_start(
                            out=wdt[:], in_=wd_d[kh * P:(kh + 1) * P, :])
                        for md in mds:
                            nc.tensor.matmul(
                                y_ps[md][:, :G],
                                lhsT=wdt[:, md * P:(md + 1) * P],
                                rhs=h_t[kh][:, :G],
                                start=(kh == 0), stop=(kh == KH - 1))
                    for md in mds:
                        ys = ysp.tile([P, G_FULL], F32, tag="ys",
                                      name=f"ys{md}")
                        nc.scalar.copy(ys[:, :G], y_ps[md][:, :G])
                        nc.sync.dma_start(
                            out=yT[md * P:(md + 1) * P, g0:g0 + G],
                            in_=ys[:, :G])

    nc.compile()
    return nc


_NC_CACHE = {}


def _get_nc(C):
    if C not in _NC_CACHE:
        _NC_CACHE[C] = build_expert_ffn(C)
    return _NC_CACHE[C]


def _route(x2d, router_w, temp):
    """fp32 router identical in selection to the jax reference."""
    t = np.clip(temp.astype(np.float32), 0.1, 5.0)
    logits = (x2d @ router_w) / t          # [T, E]
    m = logits.max(axis=-1, keepdims=True)
    ex = np.exp(logits - m)
    probs = ex / ex.sum(axis=-1, keepdims=True)
    order = np.argsort(-probs, axis=-1, kind="stable")
    idx = order[:, :TOP_K]                 # [T, 2]
    w = np.take_along_axis(probs, idx, axis=-1)
    return probs, w, idx


def kernel(x, router_w, temp, Wg, Wu, Wd):
    x = np.asarray(x, dtype=np.float32)
    router_w = np.asarray(router_w, dtype=np.float32)
    temp = np.asarray(temp, dtype=np.float32)
    Wg = np.asarray(Wg, dtype=np.float32)
    Wu = np.asarray(Wu, dtype=np.float32)
    Wd = np.asarray(Wd, dtype=np.float32)

    T = B * S
    x2d = x.reshape(T, D)
    probs, w, idx = _route(x2d, router_w, temp)

    # dispatch: token lists per expert
    tok_lists = []
    wt_lists = []
    for e in range(E):
        sel = idx == e                      # [T, 2]
        hit = sel.any(axis=1)
        toks = np.nonzero(hit)[0]
        we = (w * sel).sum(axis=1)[toks]    # combine weight per routed token
        tok_lists.append(toks)
        wt_lists.append(we.astype(np.float32))

    # Capacity factor 1.0: C = mean tokens per expert (the standard MoE
    # capacity), so the device computes exactly T*TOP_K token-expert pairs.
    # Overflow tokens of heavy experts (a few hundred, ~50ms of numpy) are
    # computed on host; if routing were ever skewed enough to exceed the
    # budget, grow C instead.
    max_load = max(len(t) for t in tok_lists)
    C_full = ((max_load + 127) // 128) * 128
    C_lite = ((T * TOP_K // E) + 127) // 128 * 128    # 2048 for this shape
    overflow_budget = 512
    while sum(max(0, len(t) - C_lite) for t in tok_lists) > overflow_budget:
        C_lite += 128
    C = min(C_full, C_lite)

    # split off host-computed overflow
    over_lists = [t[C:] for t in tok_lists]
    over_wts = [wl[C:] for wl in wt_lists]
    tok_lists = [t[:C] for t in tok_lists]
    wt_lists = [wl[:C] for wl in wt_lists]

    # per-core inputs
    in_maps = []
    for e in range(E):
        toks = tok_lists[e]
        xg = np.zeros((C, D), dtype=ml_dtypes.bfloat16)
        xg[:len(toks)] = x2d[toks]          # fp32 -> bf16 cast
        in_maps.append({
            "xT": np.ascontiguousarray(xg.T),
            "Wg": Wg[e].astype(ml_dtypes.bfloat16),
            "Wu": Wu[e].astype(ml_dtypes.bfloat16),
            "Wd": Wd[e].astype(ml_dtypes.bfloat16),
        })

    nc = _get_nc(C)
    res = None
    last_err = None
    for _attempt in range(3):
        try:
            res = run_bass_kernel_spmd(nc, in_maps,
                                       core_ids=list(range(N_CORES)))
            break
        except Exception as ex:  # transient NRT device faults happen
            last_err = ex
            import time as _time
            _time.sleep(10)
            try:  # drop the (possibly wedged) PJRT client and reconnect
                import jax
                jax.clear_caches()
                jax._src.api.clear_backends()
            except Exception:
                pass
    if res is None:
        raise last_err

    # combine in the reference's expert order
    out = np.zeros((T, D), dtype=np.float32)
    for e in range(E):
        toks = tok_lists[e]
        ye = np.asarray(res.results[e]["yT"]).T[:len(toks)]  # [n_e, D] fp32
        out[toks] += wt_lists[e][:, None] * ye
        if len(over_lists[e]):
            xo = x2d[over_lists[e]]
            gate = xo @ Wg[e]
            up = xo @ Wu[e]
            hidden = (up / (1.0 + np.exp(-up))) * gate
            out[over_lists[e]] += over_wts[e][:, None] * (hidden @ Wd[e])

    # aux losses (host, fp32; tiny)
    importance = probs.mean(axis=0)                          # [E]
    counts = np.bincount(idx.ravel(), minlength=E).astype(np.float32)
    load = counts / (np.float32(T) + np.float32(1e-6))
    aux_load_loss = np.float32((importance * load).sum() * E * 0.01)
    entropy = -(probs * np.log(np.clip(probs, 1e-8, None))).sum(axis=-1)
    router_entropy = np.float32(entropy.mean() * 0.01)
    overflow_pct = np.float32(0.0)

    return (out.reshape(B, S, D),
            np.float32(aux_load_loss),
            np.float32(router_entropy),
            np.float32(overflow_pct))
